# revision 12
# baseline (speedup 1.0000x reference)
"""BiLSTM+Chars+CRF loss kernel for trn2 (8 NeuronCores, data-parallel over batch).

Strategy
--------
- Shard batch (256) over 8 cores -> 32 sequences/core. Replicate all weights.
- Per core, one Bass/Tile module computes the full loss for its 32 sequences:
  * embedding gather via indirect DMA (token-major) + DMA-transpose into a
    feature-major x^T [115, T*32] (114 features + a ones-row folding the bias),
  * x@Wx for all gates/dirs pre-accumulated into PSUM in 8-step chunks,
  * fwd & bwd LSTM recurrences interleaved in one T-iteration loop
    (gate-major state layout [128 hidden, 32 batch]; bf16 matmuls, fp32 cell),
  * logits + CRF in a second phase: the CRF forward algorithm is evaluated in
    the *linear* domain as a product of per-step 9x9 transfer matrices
    A*diag(e_t); chunked scan (C chunks x 16 steps) where all chunks advance
    in parallel as one batched matmul against a block-diagonal constant.
    Masking (variable lengths) is handled exactly with a 2-phase (running /
    frozen) 18x18 companion matrix whose per-step column scales are m*e and
    (1-m).
- Host side does only input marshaling: sharding, transposes, one-hots, masks,
  exp(transition), index tables. All heavy float math happens on device.

Returns (log_likelihood[256] fp32, transition[9,9] fp32) like the reference.
"""

import os
import sys
import numpy as np

for _p in ("/opt/trn_rl_repo", "/root/.axon_site/_ro/trn_rl_repo"):
    if os.path.isdir(_p) and _p not in sys.path:
        sys.path.insert(0, _p)

import ml_dtypes

import concourse.bass as bass
import concourse.bacc as bacc
import concourse.mybir as mybir
import concourse.tile as tile
from concourse.bass import IndirectOffsetOnAxis

F32 = mybir.dt.float32
BF16 = mybir.dt.bfloat16
I32 = mybir.dt.int32

BF = ml_dtypes.bfloat16

# ---- problem constants ----
VOCAB = 30001
EMB = 64
CHAR = 50
RNN = 128
TAGS = 9
BFULL = 256
DIN = EMB + CHAR + 1  # 114 features + ones row (bias)
NCORES = 8
BC = BFULL // NCORES  # 32 sequences per core

CBAR = 2.75          # per-step exp centering constant (exact bookkeeping below)
NEG = -200.0         # logit offset for masked steps -> exp() == 0

# CRF chunking
LCH = 16             # steps per CRF chunk
NG = 4               # partition groups for the packed scan
PH = 2               # phases (running / frozen)
GROWS = PH * TAGS    # 18 live rows per group
GSTR = 32            # partition stride per group (engine ops need 32-aligned base)
SROWS = NG * GSTR    # 128 partitions
RENORM = 4           # renormalize alpha every RENORM chunk-combines

XCH = 8              # recurrence iterations per x@Wx PSUM chunk

MULT = mybir.AluOpType.mult
ADD = mybir.AluOpType.add
SUB = mybir.AluOpType.subtract
AXX = mybir.AxisListType.X


def mkap(base_ap, extra_off, dims):
    """Custom AP on base_ap's tensor: linear offset + dims (incl partition dim)."""
    return bass.AP(base_ap.tensor, base_ap.offset + extra_off, dims)


def build_module(T, debug=False):
    """Build the per-core Bass module."""
    NTOK = T * BC
    NTILES = NTOK // 128
    C = T // LCH                 # number of CRF chunks
    assert C % NG == 0
    CPG = C // NG                # chunks per group
    IPG = BC * CPG               # items per group
    SCOLS = IPG * TAGS           # packed state columns
    SFS = LCH * IPG              # scale_all free size
    NEV = C // RENORM            # renorm events
    HALF = SCOLS // 2
    UHALF = HALF // TAGS         # items per group-half

    nc = bacc.Bacc("TRN2", debug=False, num_devices=1)

    # ---------------- DRAM I/O ----------------
    dE = nc.dram_tensor("E", [VOCAB, EMB], F32, kind="ExternalInput").ap()
    dcharT = nc.dram_tensor("charT", [CHAR + 1, NTOK], BF16, kind="ExternalInput").ap()
    didx = nc.dram_tensor("idx", [128, NTILES], I32, kind="ExternalInput").ap()
    dwh = nc.dram_tensor("whcat", [RNN, 8 * RNN], BF16, kind="ExternalInput").ap()
    dwx = nc.dram_tensor("wxcat", [DIN, 8 * RNN], BF16, kind="ExternalInput").ap()
    dwd1 = nc.dram_tensor("wd1", [RNN, TAGS], BF16, kind="ExternalInput").ap()
    dwd2 = nc.dram_tensor("wd2", [RNN, TAGS], BF16, kind="ExternalInput").ap()
    dneg = nc.dram_tensor("negrow", [1, TAGS], BF16, kind="ExternalInput").ap()
    dmbar = nc.dram_tensor("mbar", [1, NTOK], BF16, kind="ExternalInput").ap()
    dmext = nc.dram_tensor("mextq1", [1, LCH * (T // LCH) * BC], F32, kind="ExternalInput").ap()
    doh = nc.dram_tensor("oh", [TAGS, NTOK], BF16, kind="ExternalInput").ap()
    dbinv = nc.dram_tensor("binval", [BC, T - 1], F32, kind="ExternalInput").ap()
    dgconst = nc.dram_tensor("gconst", [1, BC], F32, kind="ExternalInput").ap()
    dbdc = nc.dram_tensor("bdc", [TAGS, 1], F32, kind="ExternalInput").ap()
    dbbd = nc.dram_tensor("bbd", [SROWS, SROWS], F32, kind="ExternalInput").ap()
    dccoll = nc.dram_tensor("ccoll", [SROWS, SROWS], F32, kind="ExternalInput").ap()
    dsinit = nc.dram_tensor("sinit", [SROWS, SCOLS], F32, kind="ExternalInput").ap()
    di9 = nc.dram_tensor("i9", [TAGS, TAGS], F32, kind="ExternalInput").ap()
    di32 = nc.dram_tensor("i32", [BC, BC], F32, kind="ExternalInput").ap()

    dll = nc.dram_tensor("ll", [1, BC], F32, kind="ExternalOutput").ap()
    dbg = {}
    if debug:
        for nm, shp in [("d_alpha", [TAGS, BC]), ("d_unary", [1, BC]),
                        ("d_lnz", [1, BC]), ("d_bin", [1, BC]),
                        ("d_lnsc", [1, BC])]:
            dbg[nm] = nc.dram_tensor(nm, shp, F32, kind="ExternalOutput").ap()

    sigAF = mybir.ActivationFunctionType.Sigmoid
    tanAF = mybir.ActivationFunctionType.Tanh
    expAF = mybir.ActivationFunctionType.Exp
    logAF = mybir.ActivationFunctionType.Ln

    with tile.TileContext(nc) as tc:
        with tc.tile_pool(name="persist", bufs=1) as pp:
            # persistent SBUF
            xT = pp.tile([DIN, NTOK], BF16, name="xT")
            hf = pp.tile([RNN, (T + 1) * BC], BF16, name="hf")
            hb = pp.tile([RNN, (T + 1) * BC], BF16, name="hb")
            wh = pp.tile([RNN, 8 * RNN], BF16, name="wh")
            wx = pp.tile([DIN, 8 * RNN], BF16, name="wx")
            wd1 = pp.tile([RNN, TAGS], BF16, name="wd1")
            wd2 = pp.tile([RNN, TAGS], BF16, name="wd2")
            negw = pp.tile([1, TAGS], BF16, name="negw")
            scale_all = pp.tile([SROWS, SFS], F32, name="scale_all")
            S = pp.tile([SROWS, SCOLS], F32, name="S")
            Pcol = pp.tile([SROWS, SCOLS], F32, name="Pcol")
            upslab = pp.tile([TAGS, BC * C], F32, name="upslab")
            bdc = pp.tile([TAGS, 1], F32, name="bdc")
            bbd = pp.tile([SROWS, SROWS], F32, name="bbd")
            ccoll = pp.tile([SROWS, SROWS], F32, name="ccoll")
            i9 = pp.tile([TAGS, TAGS], F32, name="i9")
            i32t = pp.tile([BC, BC], F32, name="i32t")
            ones9 = pp.tile([TAGS, TAGS], F32, name="ones9")
            binv = pp.tile([BC, T - 1], F32, name="binv")
            gconst = pp.tile([1, BC], F32, name="gconst")
            ured = pp.tile([TAGS, BC], F32, name="ured")
            alpha_ab = [pp.tile([TAGS, BC], F32, name="alpha_a"),
                        pp.tile([TAGS, BC], F32, name="alpha_b")]
            tmp9s = [pp.tile([TAGS, BC * TAGS], F32, name="tmp9_0"),
                     pp.tile([TAGS, BC * TAGS], F32, name="tmp9_1")]
            prods = [pp.tile([TAGS, BC * TAGS], F32, name="prod_0"),
                     pp.tile([TAGS, BC * TAGS], F32, name="prod_1")]
            recs = [pp.tile([1, BC], F32, name="rec_0"),
                    pp.tile([1, BC], F32, name="rec_1")]
            scsl = pp.tile([1, BC * max(NEV, 1)], F32, name="scsl")
            scsl2 = pp.tile([1, BC * max(NEV, 1)], F32, name="scsl2")
            lnz = pp.tile([1, BC], F32, name="lnz")
            lnsc = pp.tile([1, BC], F32, name="lnsc")
            llsb = pp.tile([1, BC], F32, name="llsb")
            bred = pp.tile([BC, 1], F32, name="bred")

            # ---------------- prologue ----------------
            nc.sync.dma_start(wh[:], dwh)
            nc.sync.dma_start(wx[:], dwx)
            nc.sync.dma_start(wd1[:], dwd1)
            nc.sync.dma_start(wd2[:], dwd2)
            nc.sync.dma_start(negw[:], dneg)
            nc.sync.dma_start(bdc[:], dbdc)
            nc.sync.dma_start(bbd[:], dbbd)
            nc.sync.dma_start(ccoll[:], dccoll)
            nc.sync.dma_start(i9[:], di9)
            nc.sync.dma_start(i32t[:], di32)
            nc.sync.dma_start(binv[:], dbinv)
            nc.sync.dma_start(gconst[:], dgconst)
            nc.sync.dma_start(S[:], dsinit)
            nc.vector.memset(ones9[:], 1.0)
            nc.vector.memset(scsl[:], 1.0)

            nc.sync.dma_start(xT[EMB:DIN, :], dcharT)

            from concourse.masks import make_identity
            with tc.tile_pool(name="gather", bufs=1) as gp, \
                 tc.tile_pool(name="gpsum", bufs=4, space="PSUM") as gps:
                ident = gp.tile([128, 128], F32, name="ident")
                make_identity(nc, ident[:])
                idx_sb = gp.tile([128, NTILES], I32, name="idx_sb")
                nc.sync.dma_start(idx_sb[:], didx)
                for k in range(NTILES):
                    gt = gp.tile([128, EMB], F32, tag="gt", bufs=4)
                    nc.gpsimd.indirect_dma_start(
                        out=gt[:], out_offset=None, in_=dE,
                        in_offset=IndirectOffsetOnAxis(ap=idx_sb[:, k:k + 1], axis=0),
                    )
                    pt = gps.tile([EMB, 128], F32, tag="pt", bufs=4)
                    nc.tensor.transpose(out=pt[:], in_=gt[:], identity=ident[:])
                    nc.scalar.activation(
                        out=xT[0:EMB, k * 128:(k + 1) * 128], in_=pt[:],
                        func=mybir.ActivationFunctionType.Copy)

            nc.vector.memset(hf[:, 0:BC], 0.0)
            nc.vector.memset(hb[:, T * BC:(T + 1) * BC], 0.0)

            # ---------------- phase 1: BiLSTM ----------------
            with tc.tile_pool(name="lstm", bufs=2) as lp, \
                 tc.tile_pool(name="zpsum", bufs=2, space="PSUM") as zp:
                c_prev = lp.tile([RNN, 2 * BC], F32, tag="c", bufs=2)
                nc.vector.memset(c_prev[:], 0.0)
                GB = XCH * BC  # gate-dir block width in z
                for k in range(T // XCH):
                    z = zp.tile([RNN, 8 * GB], F32, tag="z", bufs=2)
                    zfs = 8 * GB
                    for gd in range(8):
                        if gd < 4:
                            rhs = xT[:, (XCH * k) * BC:(XCH * k + XCH) * BC]
                        else:
                            rhs = xT[:, (T - XCH - XCH * k) * BC:(T - XCH * k) * BC]
                        nc.tensor.matmul(
                            out=z[:, gd * GB:(gd + 1) * GB],
                            lhsT=wx[:, gd * RNN:(gd + 1) * RNN],
                            rhs=rhs, start=True, stop=False, skip_group_check=True)
                    for j in range(XCH):
                        i = XCH * k + j       # fwd t=i ; bwd t=T-1-i
                        for gd in range(8):
                            if gd < 4:
                                rhs = hf[:, i * BC:(i + 1) * BC]
                                oc = gd * GB + j * BC
                            else:
                                rhs = hb[:, (T - i) * BC:(T - i + 1) * BC]
                                oc = gd * GB + (XCH - 1 - j) * BC
                            nc.tensor.matmul(
                                out=z[:, oc:oc + BC],
                                lhsT=wh[:, gd * RNN:(gd + 1) * RNN],
                                rhs=rhs, start=False, stop=True,
                                skip_group_check=True)
                        dstep = 4 * GB + (XCH - 1 - 2 * j) * BC  # bwd col - fwd col
                        base = j * BC
                        sg = lp.tile([RNN, 6 * BC], BF16, tag="sg", bufs=2)
                        nc.scalar.activation(
                            out=mkap(sg[:], 0, [[6 * BC, RNN], [3 * BC, 2], [BC, 3], [1, BC]]),
                            in_=mkap(z[:], base, [[zfs, RNN], [dstep, 2], [GB, 3], [1, BC]]),
                            func=sigAF)
                        tg = lp.tile([RNN, 2 * BC], BF16, tag="tg", bufs=2)
                        nc.scalar.activation(
                            out=mkap(tg[:], 0, [[2 * BC, RNN], [BC, 2], [1, BC]]),
                            in_=mkap(z[:], 3 * GB + base, [[zfs, RNN], [dstep, 2], [1, BC]]),
                            func=tanAF)

                        def sgv(gate):  # sigmoid-gate view, shape (RNN, 2, BC)
                            return mkap(sg[:], gate * BC,
                                        [[6 * BC, RNN], [3 * BC, 2], [1, BC]])
                        db_ = [[2 * BC, RNN], [BC, 2], [1, BC]]
                        t1 = lp.tile([RNN, 2 * BC], F32, tag="t1", bufs=2)
                        nc.vector.tensor_tensor(
                            out=mkap(t1[:], 0, db_), in0=sgv(0),
                            in1=mkap(tg[:], 0, db_), op=MULT)
                        c_new = lp.tile([RNN, 2 * BC], F32, tag="c", bufs=2)
                        nc.vector.tensor_tensor(
                            out=mkap(c_new[:], 0, db_), in0=sgv(1),
                            in1=mkap(c_prev[:], 0, db_), op=MULT)
                        nc.vector.tensor_tensor(
                            out=c_new[:], in0=c_new[:], in1=t1[:], op=ADD)
                        c_prev = c_new
                        th = lp.tile([RNN, 2 * BC], BF16, tag="th", bufs=2)
                        nc.scalar.activation(out=th[:], in_=c_new[:], func=tanAF)
                        nc.vector.tensor_tensor(
                            out=hf[:, (i + 1) * BC:(i + 2) * BC],
                            in0=mkap(sg[:], 2 * BC, [[6 * BC, RNN], [1, BC]]),
                            in1=th[:, 0:BC], op=MULT)
                        nc.vector.tensor_tensor(
                            out=hb[:, (T - 1 - i) * BC:(T - i) * BC],
                            in0=mkap(sg[:], 5 * BC, [[6 * BC, RNN], [1, BC]]),
                            in1=th[:, BC:2 * BC], op=MULT)

            # ---------------- phase 2a: logits, exp -> scale_all, unary ----------------
            nc.gpsimd.memset(scale_all[:], 0.0)  # dead group rows must be 0
            alpha = alpha_ab[0]
            with tc.tile_pool(name="lg", bufs=2, space="PSUM") as lgp, \
                 tc.tile_pool(name="u2a", bufs=2) as up:
                for kk in range(C):
                    W = LCH * BC
                    lg = lgp.tile([TAGS, W], F32, tag="lg", bufs=2)
                    mbc = up.tile([1, W], BF16, tag="mbc", bufs=2)
                    nc.sync.dma_start(mbc[:], dmbar[:, LCH * kk * BC:LCH * (kk + 1) * BC])
                    ohc = up.tile([TAGS, W], BF16, tag="ohc", bufs=2)
                    nc.sync.dma_start(ohc[:], doh[:, LCH * kk * BC:LCH * (kk + 1) * BC])
                    nc.tensor.matmul(out=lg[:], lhsT=wd1[:],
                                     rhs=hf[:, (LCH * kk + 1) * BC:(LCH * (kk + 1) + 1) * BC],
                                     start=True, stop=False, skip_group_check=True)
                    nc.tensor.matmul(out=lg[:], lhsT=wd2[:],
                                     rhs=hb[:, LCH * kk * BC:LCH * (kk + 1) * BC],
                                     start=False, stop=False, skip_group_check=True)
                    nc.tensor.matmul(out=lg[:], lhsT=negw[:],
                                     rhs=mbc[:],
                                     start=False, stop=True, skip_group_check=True)
                    ltc = up.tile([TAGS, W], BF16, tag="ltc", bufs=2)
                    nc.vector.tensor_tensor(out=ltc[:], in0=lg[:], in1=ohc[:], op=MULT)
                    nc.vector.tensor_reduce(
                        out=upslab[:, kk * BC:(kk + 1) * BC],
                        in_=mkap(ltc[:], 0, [[W, TAGS], [1, BC], [BC, LCH]]),
                        axis=AXX, op=ADD)
                    g, u0 = kk // CPG, (kk % CPG) * BC
                    nc.scalar.activation(
                        out=mkap(scale_all[:], (GSTR * g) * SFS + u0,
                                 [[SFS, TAGS], [IPG, LCH - 1], [1, BC]]),
                        in_=mkap(lg[:], BC, [[W, TAGS], [BC, LCH - 1], [1, BC]]),
                        func=expAF, bias=bdc[:], scale=1.0)
                    inb = mkap(lg[:], 0, [[W, TAGS], [1, BC]])
                    if kk == 0:
                        nc.scalar.activation(out=alpha[:], in_=inb, func=expAF,
                                             bias=bdc[:], scale=1.0)
                    else:
                        gp_, up_ = (kk - 1) // CPG, ((kk - 1) % CPG) * BC
                        nc.scalar.activation(
                            out=mkap(scale_all[:],
                                     (GSTR * gp_) * SFS + (LCH - 1) * IPG + up_,
                                     [[SFS, TAGS], [1, BC]]),
                            in_=inb, func=expAF, bias=bdc[:], scale=1.0)
            # pad slot t=T (chunk C-1, s=LCH-1): zero
            nc.vector.memset(
                mkap(scale_all[:], (GSTR * (NG - 1)) * SFS
                     + (LCH - 1) * IPG + (CPG - 1) * BC,
                     [[SFS, TAGS], [1, BC]]), 0.0)
            # q1 rows: (1-m) from host, pre-arranged in scale_all column order
            for g in range(NG):
                dst = mkap(scale_all[:], (GSTR * g + TAGS) * SFS,
                           [[SFS, TAGS], [1, SFS]])
                srcap = mkap(dmext, g * SFS, [[0, TAGS], [1, SFS]])
                nc.sync.dma_start(dst, srcap)

            # unary reduction over chunks
            nc.vector.tensor_reduce(
                out=ured[:],
                in_=mkap(upslab[:], 0, [[BC * C, TAGS], [1, BC], [BC, C]]),
                axis=AXX, op=ADD)

            # ---------------- phase 2b: packed transfer-matrix scan ----------------
            with tc.tile_pool(name="sp", bufs=1, space="PSUM") as sp:
                for s in range(LCH):
                    for h_ in range(2):
                        c0 = h_ * HALF
                        mo = sp.tile([SROWS, HALF], F32, tag=f"mo{h_}", bufs=1)
                        for seg in range(0, HALF, 512):
                            w = min(512, HALF - seg)
                            nc.tensor.matmul(out=mo[:, seg:seg + w], lhsT=bbd[:],
                                             rhs=S[:, c0 + seg:c0 + seg + w],
                                             start=True, stop=True)
                        nc.vector.tensor_tensor(
                            out=mkap(S[:], c0, [[SCOLS, SROWS], [TAGS, UHALF], [1, TAGS]]),
                            in0=mkap(mo[:], 0, [[HALF, SROWS], [TAGS, UHALF], [1, TAGS]]),
                            in1=mkap(scale_all[:], s * IPG + h_ * UHALF,
                                     [[SFS, SROWS], [1, UHALF], [0, TAGS]]),
                            op=MULT)
            # collapse phases: Pcol = ccoll^T @ S
            with tc.tile_pool(name="cp", bufs=1, space="PSUM") as cp:
                po = cp.tile([SROWS, SCOLS], F32, name="po")
                for seg in range(0, SCOLS, 512):
                    w = min(512, SCOLS - seg)
                    nc.tensor.matmul(out=po[:, seg:seg + w], lhsT=ccoll[:],
                                     rhs=S[:, seg:seg + w], start=True, stop=True)
                nc.vector.tensor_copy(out=Pcol[:], in_=po[:])

            # ---------------- phase 2d: combine chunks + finals ----------------
            with tc.tile_pool(name="cb", bufs=1, space="PSUM") as cb:
                aexps = [cb.tile([TAGS, BC * TAGS], F32, name="aexp0"),
                         cb.tile([TAGS, BC * TAGS], F32, name="aexp1")]
                rows = [cb.tile([1, BC], F32, name=f"row{i_}") for i_ in range(3)]
                rec9 = cb.tile([TAGS, BC], F32, name="rec9")
                ev = 0
                for c in range(C):
                    g, u0 = c // CPG, (c % CPG) * BC
                    alpha_n = alpha_ab[(c + 1) % 2]
                    tmp9 = tmp9s[c % 2]
                    nc.vector.tensor_tensor(
                        out=mkap(tmp9[:], 0, [[BC * TAGS, TAGS], [TAGS, BC], [1, TAGS]]),
                        in0=mkap(alpha[:], 0, [[BC, TAGS], [1, BC], [0, TAGS]]),
                        in1=mkap(i9[:], 0, [[TAGS, TAGS], [0, BC], [1, TAGS]]),
                        op=MULT)
                    aexp = aexps[c % 2]
                    nc.tensor.matmul(out=aexp[:], lhsT=ones9[:], rhs=tmp9[:],
                                     start=True, stop=True)
                    prod = prods[c % 2]
                    nc.vector.tensor_tensor(
                        out=prod[:], in0=aexp[:],
                        in1=mkap(Pcol[:], (GSTR * g) * SCOLS + u0 * TAGS,
                                 [[SCOLS, TAGS], [1, BC * TAGS]]),
                        op=MULT)
                    nc.vector.tensor_reduce(
                        out=alpha_n[:],
                        in_=mkap(prod[:], 0, [[BC * TAGS, TAGS], [TAGS, BC], [1, TAGS]]),
                        axis=AXX, op=ADD)
                    alpha = alpha_n
                    if (c + 1) % RENORM == 0 and NEV > 0:
                        ssum = rows[ev % 3]
                        nc.tensor.matmul(out=ssum[:], lhsT=ones9[:, 0:1],
                                         rhs=alpha[:], start=True, stop=True)
                        nc.vector.tensor_copy(out=scsl[:, ev * BC:(ev + 1) * BC],
                                              in_=ssum[:])
                        rec = recs[ev % 2]
                        nc.vector.reciprocal(out=rec[:], in_=ssum[:])
                        nc.tensor.matmul(out=rec9[:], lhsT=ones9[0:1, :],
                                         rhs=rec[:], start=True, stop=True)
                        nc.vector.tensor_tensor(out=alpha[:], in0=alpha[:],
                                                in1=rec9[:], op=MULT)
                        ev += 1

                # ---------------- phase 3: final assembly ----------------
                asum = rows[0]
                nc.tensor.matmul(out=asum[:], lhsT=ones9[:, 0:1], rhs=alpha[:],
                                 start=True, stop=True)
                nc.scalar.activation(out=lnz[:], in_=asum[:], func=logAF)
                nc.scalar.activation(out=scsl2[:], in_=scsl[:], func=logAF)
                if NEV > 0:
                    nc.vector.tensor_reduce(
                        out=lnsc[:],
                        in_=mkap(scsl2[:], 0, [[BC * NEV, 1], [1, BC], [BC, NEV]]),
                        axis=AXX, op=ADD)
                else:
                    nc.vector.memset(lnsc[:], 0.0)
                usum = rows[1]
                nc.tensor.matmul(out=usum[:], lhsT=ones9[:, 0:1], rhs=ured[:],
                                 start=True, stop=True)
                nc.vector.tensor_reduce(out=bred[:], in_=binv[:], axis=AXX, op=ADD)
                brow = rows[2]
                nc.tensor.matmul(out=brow[:], lhsT=bred[:], rhs=i32t[:],
                                 start=True, stop=True)
                nc.vector.tensor_copy(out=llsb[:], in_=usum[:])
                nc.vector.tensor_tensor(out=llsb[:], in0=llsb[:], in1=brow[:], op=ADD)
                nc.vector.tensor_tensor(out=llsb[:], in0=llsb[:], in1=gconst[:], op=ADD)
                nc.vector.tensor_tensor(out=llsb[:], in0=llsb[:], in1=lnz[:], op=SUB)
                nc.vector.tensor_tensor(out=llsb[:], in0=llsb[:], in1=lnsc[:], op=SUB)
                nc.sync.dma_start(dll, llsb[:])
                if debug:
                    nc.sync.dma_start(dbg["d_alpha"], alpha[:])
                    dbg_u = pp.tile([1, BC], F32, name="dbg_u")
                    nc.vector.tensor_copy(out=dbg_u[:], in_=usum[:])
                    nc.sync.dma_start(dbg["d_unary"], dbg_u[:])
                    nc.sync.dma_start(dbg["d_lnz"], lnz[:])
                    dbg_b = pp.tile([1, BC], F32, name="dbg_b")
                    nc.vector.tensor_copy(out=dbg_b[:], in_=brow[:])
                    nc.sync.dma_start(dbg["d_bin"], dbg_b[:])
                    nc.sync.dma_start(dbg["d_lnsc"], lnsc[:])

    nc.compile()
    return nc


# ---------------- host-side preparation ----------------

def prep_core_inputs(core, inputs, char_embeddings, labels, E, Wx_f, Wh_f, b_f,
                     Wx_b, Wh_b, b_b, Wd, bd, transition, T):
    NTOK = T * BC
    NTILES = NTOK // 128
    C = T // LCH
    CPG = C // NG
    IPG = BC * CPG
    SCOLS = IPG * TAGS

    sl = slice(core * BC, (core + 1) * BC)
    idx = np.asarray(inputs[sl, :T]).astype(np.int32)        # [BC, T]
    ch = np.asarray(char_embeddings[sl, :T]).astype(np.float32)
    lab = np.asarray(labels[sl, :T]).astype(np.int64)

    lengths = (lab != 0).sum(axis=1)                          # [BC]
    mask = (np.arange(T)[None, :] < lengths[:, None])         # [BC, T] bool

    idx_tm = idx.T.reshape(-1)                                # token n = t*BC+b
    idx_tiles = np.ascontiguousarray(idx_tm.reshape(NTILES, 128).T)
    charT = np.concatenate(
        [ch.transpose(2, 1, 0).reshape(CHAR, NTOK),
         np.ones((1, NTOK), np.float32)], axis=0).astype(BF)

    perm = [0, 1, 3, 2]  # keras (i,f,g,o) -> ours (i,f,o,g)

    def gates(W):
        W = np.asarray(W, np.float32)
        one_d = W.ndim == 1
        if one_d:
            W = W[None, :]
        W = W.reshape(W.shape[0], 4, RNN)[:, perm, :].reshape(W.shape[0], 4 * RNN)
        return W

    whcat = np.concatenate([gates(Wh_f), gates(Wh_b)], axis=1).astype(BF)
    wxf = np.concatenate([gates(Wx_f), gates(b_f)], axis=0)
    wxb = np.concatenate([gates(Wx_b), gates(b_b)], axis=0)
    wxcat = np.concatenate([wxf, wxb], axis=1).astype(BF)     # [115, 1024]

    Wd_ = np.asarray(Wd, np.float32)
    bd_ = np.asarray(bd, np.float32)
    tr = np.asarray(transition, np.float32)

    mbar_f = (1.0 - mask.astype(np.float32))
    mbar_row = np.ascontiguousarray(mbar_f.T.reshape(1, NTOK))
    mbar_ext = np.concatenate([mbar_f, np.ones((BC, 1), np.float32)], 1)  # [BC, T+1]
    g_, s_, cp_ = np.meshgrid(np.arange(NG), np.arange(LCH), np.arange(CPG),
                              indexing="ij")
    tgrid = 1 + LCH * (CPG * g_ + cp_) + s_                  # [NG, LCH, CPG]
    mextq1 = np.ascontiguousarray(
        mbar_ext[:, tgrid].transpose(1, 2, 3, 0).reshape(1, -1))

    ohm = np.eye(TAGS, dtype=np.float32)[lab] * mask[:, :, None]
    oh = np.ascontiguousarray(ohm.transpose(2, 1, 0).reshape(TAGS, NTOK)).astype(BF)

    binval = (tr[lab[:, :-1], lab[:, 1:]] * mask[:, 1:]).astype(np.float32)
    goldbd = (bd_[lab] * mask).sum(axis=1)
    gconst = np.ascontiguousarray(
        (goldbd - CBAR * lengths.astype(np.float32)).reshape(1, BC))

    bdc = (bd_ - CBAR).reshape(TAGS, 1).astype(np.float32)

    A = np.exp(tr)
    Bt = np.zeros((GROWS, GROWS), np.float32)
    Bt[:TAGS, :TAGS] = A
    Bt[:TAGS, TAGS:] = np.eye(TAGS)
    Bt[TAGS:, TAGS:] = np.eye(TAGS)
    bbd = np.zeros((SROWS, SROWS), np.float32)
    ccoll = np.zeros((SROWS, SROWS), np.float32)
    for g in range(NG):
        bbd[g * GSTR:g * GSTR + GROWS, g * GSTR:g * GSTR + GROWS] = Bt
        ccoll[g * GSTR:g * GSTR + TAGS, g * GSTR:g * GSTR + TAGS] = np.eye(TAGS)
        ccoll[g * GSTR + TAGS:g * GSTR + GROWS, g * GSTR:g * GSTR + TAGS] = np.eye(TAGS)

    sinit = np.zeros((SROWS, SCOLS), np.float32)
    eye_cols = np.tile(np.eye(TAGS, dtype=np.float32)[None], (IPG, 1, 1))
    for g in range(NG):
        sinit[g * GSTR:g * GSTR + TAGS] = \
            eye_cols.transpose(2, 0, 1).reshape(TAGS, SCOLS)

    return {
        "E": np.asarray(E, np.float32),
        "charT": charT, "idx": idx_tiles,
        "whcat": whcat, "wxcat": wxcat,
        "wd1": Wd_[:RNN].astype(BF), "wd2": Wd_[RNN:].astype(BF),
        "negrow": np.full((1, TAGS), NEG, np.float32).astype(BF),
        "mbar": mbar_row.astype(BF), "mextq1": mextq1,
        "oh": oh, "binval": binval, "gconst": gconst, "bdc": bdc,
        "bbd": bbd, "ccoll": ccoll, "sinit": sinit,
        "i9": np.eye(TAGS, dtype=np.float32),
        "i32": np.eye(BC, dtype=np.float32),
    }


_CACHE = {}


def _get_nc(T, debug=False):
    key = (T, debug)
    if key not in _CACHE:
        _CACHE[key] = build_module(T=T, debug=debug)
    return _CACHE[key]


def _prepare(inputs, char_embeddings, labels, E, Wx_f, Wh_f, b_f, Wx_b, Wh_b,
             b_b, Wd, bd, transition):
    T = int(np.asarray(inputs).shape[1])
    nc = _get_nc(T=T)
    in_maps = [prep_core_inputs(c, inputs, char_embeddings, labels, E, Wx_f,
                                Wh_f, b_f, Wx_b, Wh_b, b_b, Wd, bd, transition,
                                T=T)
               for c in range(NCORES)]
    return nc, in_maps


def kernel(inputs, char_embeddings, labels, E, Wx_f, Wh_f, b_f, Wx_b, Wh_b, b_b,
           Wd, bd, transition, trace=False):
    from concourse.bass_utils import run_bass_kernel_spmd
    nc, in_maps = _prepare(inputs, char_embeddings, labels, E, Wx_f, Wh_f, b_f,
                           Wx_b, Wh_b, b_b, Wd, bd, transition)
    res = run_bass_kernel_spmd(nc, in_maps, core_ids=list(range(NCORES)),
                               trace=trace)
    ll = np.concatenate([np.asarray(res.results[c]["ll"]).reshape(BC)
                         for c in range(NCORES)])
    out = (ll.astype(np.float32), np.asarray(transition, np.float32))
    if trace:
        return out, res
    return out
